# revision 6
# baseline (speedup 1.0000x reference)
"""Trainium2 Bass kernel for the NodeEdge GNN message-passing module.

Computes  out[b,n,h] = sum_e (w*inci + b)[n,e] * relu(inputs @ W_xes + b_xes)[b,e,h]
with B=16, N=2048, E=8192, DIM=64, DH=32.

Strategy: shard the edge (contraction) dimension E across the 8 NeuronCores
(EC=1024 edges per core); partial outputs are summed on the host.

The whole on-device datapath is bf16 (correctness gate is rel_err < 2e-2;
bf16 quantization lands ~4e-3):
  - inputs / w / W_xes ship as bf16, inci ships as uint8 and is cast to
    bf16 inside the SWDGE DMA (gpsimd-initiated DMAs cast in the DMA
    datapath, so no compute engine touches the raw u8),
  - xe = relu(inputs @ W_xes) via PE matmuls, relu on ScalarE casting
    PSUM f32 -> bf16,
  - A = w*inci via one 2x-mode DVE multiply per 128-row e-chunk,
  - big matmul out[(b,h), n] += xe^T @ A^T in bf16, f32 PSUM accum,
  - output partials stored as bf16.

The big matmul needs 16 PSUM-bank-sized accumulators but TRN2 has 8
banks, so banks are time-shared in three stages with partial parking:
  stage 1: (b,h)-chunks 0-1 accumulate e-chunks 0-3 as they arrive,
           then the 8 partials park to SBUF (f32), freeing the banks;
  stage 2: (b,h)-chunks 2-3 run their full 8-chunk chains — e-chunks
           0-3 replay dense from SBUF while 4-7 stream in;
  stage 3: (b,h)-chunks 0-1 accumulate e-chunks 4-7; the parked
           partial is added back during the PSUM->SBUF evacuation.
This keeps the PE busy during the DMA stream and leaves only 32 (not
64) matmuls after the last chunk lands.

DMA queue placement is deliberate: inp tiles and the first half of the
A^T chunks interleave on the sync queue, the rest of the A^T chunks go
on the scalar queue, inci on the gpsimd queue — so the first chunk's
operands and all xe inputs land within the first few microseconds of
DMA time instead of serializing behind each other.
"""

from contextlib import ExitStack

import ml_dtypes
import numpy as np

import concourse.bass as bass
import concourse.mybir as mybir
import concourse.tile as tile
from concourse import bacc
from concourse.bass_utils import run_bass_kernel_spmd

B, N, E, DIM = 16, 2048, 8192, 64
DH = DIM // 2              # 32
NCORES = 8
EC = E // NCORES           # 1024 edges per core
KC = EC // 128             # 8 e-chunks of 128
BH = B * DH                # 512 (flattened (b, h) output dim)
NB = N // 512              # 4 column blocks of the big matmul
NJ = B // 2                # 8 input tiles, two batch rows packed per tile
KSPLIT = KC // 2           # stage-1/stage-3 split of the contraction

F32 = mybir.dt.float32
BF16 = mybir.dt.bfloat16
U8 = mybir.dt.uint8
BF16NP = ml_dtypes.bfloat16

_PROGRAMS: dict = {}


def _build_program(with_bxes: bool, with_b: bool):
    nc = bacc.Bacc(
        "TRN2", target_bir_lowering=False, debug=False, enable_asserts=False
    )

    inp_t = nc.dram_tensor("inp_t", [NJ, 128, EC], BF16, kind="ExternalInput").ap()
    wq = nc.dram_tensor("wq", [KC, 128, N], BF16, kind="ExternalInput").ap()
    iq = nc.dram_tensor("iq", [KC, 128, N], U8, kind="ExternalInput").ap()
    wx = nc.dram_tensor("wx", [128, 2 * DH], BF16, kind="ExternalInput").ap()
    bxr = (
        nc.dram_tensor("bxr", [128, BH], F32, kind="ExternalInput").ap()
        if with_bxes
        else None
    )
    bq = (
        nc.dram_tensor("bq", [KC, 128, N], BF16, kind="ExternalInput").ap()
        if with_b
        else None
    )
    outp = nc.dram_tensor("outp", [BH, N], BF16, kind="ExternalOutput").ap()

    with tile.TileContext(nc) as tc, ExitStack() as ctx:
        inp_pool = ctx.enter_context(tc.tile_pool(name="inp", bufs=NJ))
        wx_pool = ctx.enter_context(tc.tile_pool(name="wx", bufs=1))
        xe_pool = ctx.enter_context(tc.tile_pool(name="xe", bufs=KC))
        a_pool = ctx.enter_context(tc.tile_pool(name="a", bufs=KC))
        i_pool = ctx.enter_context(tc.tile_pool(name="i", bufs=4))
        park_pool = ctx.enter_context(tc.tile_pool(name="pk", bufs=2 * NB))
        out_pool = ctx.enter_context(tc.tile_pool(name="o", bufs=4))
        ps_pool = ctx.enter_context(tc.tile_pool(name="ps", bufs=8, space="PSUM"))

        # Block-diagonal xes weight: rows 0-63 map the even batch row to
        # output cols 0-31, rows 64-127 map the odd batch row to cols
        # 32-63, so one K=128 matmul computes xe for both packed batch
        # rows of an input tile at once.
        wx_tile = wx_pool.tile([128, 2 * DH], BF16)
        nc.sync.dma_start(wx_tile[:], wx[:])

        bx_tile = None
        if with_bxes:
            bx_tile = wx_pool.tile([128, BH], F32, tag="bx")
            nc.sync.dma_start(bx_tile[:], bxr[:])

        # ---- HAM warmup: a short matmul burst during the initial DMA
        # dead time (only wx has landed) so the PE clock-gate sees
        # activity early. Writes a scratch region of the psum bank that
        # xe chunk 7 will reuse much later.
        ps_warm = ps_pool.tile([128, BH], F32, tag="ps", name="ps_warm")
        for i in range(8):
            nc.tensor.matmul(
                ps_warm[0:64, 0:64],
                wx_tile[:, 0:64],
                wx_tile[:, 0:64],
                start=True,
                stop=True,
            )

        # ---- loads. Per-engine FIFO order is the scheduling knob:
        #   sync:   inp j0-3, wq0, inp j4-7, wq1-3   (xe inputs + first chunks)
        #   scalar: wq4-7                            (late chunks, own queue)
        #   gpsimd: iq0-7                            (SWDGE, casts u8->bf16)
        inp_tiles = [
            inp_pool.tile([128, EC], BF16, tag="inp", name=f"inp_{j}", bufs=NJ)
            for j in range(NJ)
        ]
        a_tiles = [
            a_pool.tile([128, N], BF16, tag="a", name=f"a_{k}", bufs=KC)
            for k in range(KC)
        ]
        it_tiles = [
            i_pool.tile([128, N], BF16, tag="it", name=f"it_{k}", bufs=4)
            for k in range(KC)
        ]

        for j in range(4):
            nc.sync.dma_start(inp_tiles[j][:], inp_t[j])
        nc.sync.dma_start(a_tiles[0][:], wq[0])
        for j in range(4, NJ):
            nc.sync.dma_start(inp_tiles[j][:], inp_t[j])
        for k in range(1, 4):
            nc.sync.dma_start(a_tiles[k][:], wq[k])
        for k in range(4, KC):
            nc.scalar.dma_start(a_tiles[k][:], wq[k])
        for k in range(KC):
            nc.gpsimd.dma_start(it_tiles[k][:], iq[k])

        # ---- xe = relu(inputs @ W_xes) in [e, (b,h)] layout.
        # One PSUM bank per e-chunk; walk j outermost so each input tile
        # is consumed as soon as its DMA lands.
        ps_xe = [
            ps_pool.tile([128, BH], F32, tag="ps", name=f"ps_xe_{k}")
            for k in range(KC)
        ]
        for j in range(NJ):
            for k in range(KC):
                nc.tensor.matmul(
                    ps_xe[k][:, j * 2 * DH : (j + 1) * 2 * DH],
                    inp_tiles[j][:, k * 128 : (k + 1) * 128],
                    wx_tile[:],
                    start=True,
                    stop=True,
                )
        xe_tiles = []
        for k in range(KC):
            xt = xe_pool.tile([128, BH], BF16)
            if with_bxes:
                nc.vector.tensor_tensor(
                    xt[:], ps_xe[k][:], bx_tile[:], op=mybir.AluOpType.add
                )
                nc.scalar.activation(
                    xt[:], xt[:], mybir.ActivationFunctionType.Relu
                )
            else:
                nc.scalar.activation(
                    xt[:], ps_xe[k][:], mybir.ActivationFunctionType.Relu
                )
            xe_tiles.append(xt)

        # ---- A^T chunks: a[k] *= inci[k] (one full-width 2x-mode DVE
        # multiply per chunk).
        for k in range(KC):
            nc.vector.tensor_tensor(
                a_tiles[k][:], a_tiles[k][:], it_tiles[k][:],
                op=mybir.AluOpType.mult,
            )
            if with_b:
                bt = i_pool.tile([128, N], BF16, tag="bt", bufs=2)
                nc.sync.dma_start(bt[:], bq[k])
                nc.vector.tensor_tensor(
                    a_tiles[k][:], a_tiles[k][:], bt[:],
                    op=mybir.AluOpType.add,
                )

        # ---- big matmul: out[(b,h), n] += xe^T @ A^T, bf16, f32 accum.

        # stage 1: (b,h)-chunks 0-1, e-chunks 0..KSPLIT-1 as they arrive.
        ps1 = [
            [
                ps_pool.tile([128, 512], F32, tag="ps", name=f"ps1_{h}_{nb}")
                for nb in range(NB)
            ]
            for h in range(2)
        ]
        for k in range(KSPLIT):
            for h in range(2):
                lhsT = xe_tiles[k][:, h * 128 : (h + 1) * 128]
                for nb in range(NB):
                    nc.tensor.matmul(
                        ps1[h][nb][:],
                        lhsT,
                        a_tiles[k][:, nb * 512 : (nb + 1) * 512],
                        start=(k == 0),
                        stop=(k == KSPLIT - 1),
                    )
        # park the 8 stage-1 partials to SBUF (f32), freeing the banks.
        # All parks go on ScalarE: the DVE is busy with the arriving
        # chunks' mask-multiplies at this point, and the stage-3
        # evacuation adds can only run on the DVE (ScalarE has no
        # tensor_tensor).
        park = [[None] * NB for _ in range(2)]
        for h in range(2):
            for nb in range(NB):
                pk = park_pool.tile([128, 512], F32, tag="pk", name=f"pk_{h}_{nb}")
                nc.scalar.activation(
                    pk[:], ps1[h][nb][:],
                    mybir.ActivationFunctionType.Identity,
                )
                park[h][nb] = pk

        # stage 2: (b,h)-chunks 2-3 full chains; e-chunks 0..3 replay
        # dense from SBUF, 4..7 consumed as they arrive.
        ps2 = [
            [
                ps_pool.tile([128, 512], F32, tag="ps", name=f"ps2_{h}_{nb}")
                for nb in range(NB)
            ]
            for h in range(2)
        ]
        for k in range(KC):
            for h in range(2):
                lhsT = xe_tiles[k][:, (h + 2) * 128 : (h + 3) * 128]
                for nb in range(NB):
                    nc.tensor.matmul(
                        ps2[h][nb][:],
                        lhsT,
                        a_tiles[k][:, nb * 512 : (nb + 1) * 512],
                        start=(k == 0),
                        stop=(k == KC - 1),
                    )
        for h in range(2):
            ot = out_pool.tile([128, N], BF16, tag="o", name=f"ot2_{h}")
            for nb in range(NB):
                if nb % 2 == 0:
                    nc.scalar.activation(
                        ot[:, nb * 512 : (nb + 1) * 512],
                        ps2[h][nb][:],
                        mybir.ActivationFunctionType.Identity,
                    )
                else:
                    nc.vector.tensor_copy(
                        ot[:, nb * 512 : (nb + 1) * 512], ps2[h][nb][:]
                    )
            nc.scalar.dma_start(outp[(h + 2) * 128 : (h + 3) * 128, :], ot[:])

        # stage 3: (b,h)-chunks 0-1, e-chunks KSPLIT..KC-1; the parked
        # stage-1 partial is added back during evacuation.
        ps3 = [
            [
                ps_pool.tile([128, 512], F32, tag="ps", name=f"ps3_{h}_{nb}")
                for nb in range(NB)
            ]
            for h in range(2)
        ]
        for k in range(KSPLIT, KC):
            for h in range(2):
                lhsT = xe_tiles[k][:, h * 128 : (h + 1) * 128]
                for nb in range(NB):
                    nc.tensor.matmul(
                        ps3[h][nb][:],
                        lhsT,
                        a_tiles[k][:, nb * 512 : (nb + 1) * 512],
                        start=(k == KSPLIT),
                        stop=(k == KC - 1),
                    )
        for h in range(2):
            ot = out_pool.tile([128, N], BF16, tag="o", name=f"ot3_{h}")
            for nb in range(NB):
                nc.vector.tensor_tensor(
                    ot[:, nb * 512 : (nb + 1) * 512],
                    ps3[h][nb][:],
                    park[h][nb][:],
                    op=mybir.AluOpType.add,
                )
            nc.sync.dma_start(outp[h * 128 : (h + 1) * 128, :], ot[:])

    nc.compile()
    return nc


def _get_program(with_bxes: bool, with_b: bool):
    key = (with_bxes, with_b)
    if key not in _PROGRAMS:
        _PROGRAMS[key] = _build_program(with_bxes, with_b)
    return _PROGRAMS[key]


def _prepare_in_maps(inputs, W_xes, b_xes, inci, w, b, with_bxes, with_b):
    inputs = np.asarray(inputs, dtype=np.float32)
    W_xes = np.asarray(W_xes, dtype=np.float32)
    b_xes = np.asarray(b_xes, dtype=np.float32)
    w = np.asarray(w, dtype=np.float32)
    b = np.asarray(b, dtype=np.float32)
    inci_u8 = np.asarray(inci).astype(np.uint8)

    wx_dup = np.zeros((128, 2 * DH), dtype=np.float32)
    wx_dup[0:DIM, 0:DH] = W_xes
    wx_dup[DIM : 2 * DIM, DH : 2 * DH] = W_xes
    wx_dup = wx_dup.astype(BF16NP)
    bxr = np.ascontiguousarray(
        np.broadcast_to(np.tile(b_xes, B)[None, :], (128, BH))
    ) if with_bxes else None

    in_maps = []
    for c in range(NCORES):
        sl = slice(c * EC, (c + 1) * EC)
        t = np.ascontiguousarray(
            inputs[:, sl, :].transpose(0, 2, 1)
        ).reshape(NJ, 128, EC).astype(BF16NP)
        wq_ = np.ascontiguousarray(w[:, sl].T).reshape(KC, 128, N).astype(BF16NP)
        iq_ = np.ascontiguousarray(inci_u8[:, sl].T).reshape(KC, 128, N)
        m = {"inp_t": t, "wq": wq_, "iq": iq_, "wx": wx_dup}
        if with_bxes:
            m["bxr"] = bxr
        if with_b:
            m["bq"] = np.ascontiguousarray(b[:, sl].T).reshape(
                KC, 128, N
            ).astype(BF16NP)
        in_maps.append(m)
    return in_maps


def _run(inputs, W_xes, b_xes, inci, w, b, **run_kwargs):
    with_bxes = bool(np.any(np.asarray(b_xes)))
    with_b = bool(np.any(np.asarray(b)))
    nc = _get_program(with_bxes, with_b)
    in_maps = _prepare_in_maps(inputs, W_xes, b_xes, inci, w, b, with_bxes, with_b)
    res = run_bass_kernel_spmd(
        nc, in_maps, core_ids=list(range(NCORES)), **run_kwargs
    )
    parts = np.stack(
        [r["outp"].astype(np.float32) for r in res.results]
    )  # [8, BH, N]
    out = parts.sum(axis=0)  # [BH, N]
    out = out.reshape(B, DH, N).transpose(0, 2, 1)  # [B, N, DH]
    return np.ascontiguousarray(out.astype(np.float32)), res


def kernel(inputs, W_xes, b_xes, inci, w, b):
    out, _ = _run(inputs, W_xes, b_xes, inci, w, b)
    return out


# revision 7
# speedup vs baseline: 1.3028x; 1.3028x over previous
"""Trainium2 Bass kernel for the NodeEdge GNN message-passing module.

Computes  out[b,n,h] = sum_e (w*inci + b)[n,e] * relu(inputs @ W_xes + b_xes)[b,e,h]
with B=16, N=2048, E=8192, DIM=64, DH=32.

Strategy: shard the edge (contraction) dimension E across the 8 NeuronCores
(EC=1024 edges per core); partial outputs are summed on the host.

The whole on-device datapath is bf16 (correctness gate is rel_err < 2e-2;
bf16 quantization lands ~4e-3):
  - inputs / w / W_xes ship as bf16; inci ships as raw uint8 and is
    consumed directly by the DVE mask-multiply (mixed-dtype
    tensor_tensor; the DVE converts operands to fp32 internally),
  - xe = relu(inputs @ W_xes) via PE matmuls, relu on ScalarE casting
    PSUM f32 -> bf16,
  - big matmul out[(b,h), n] += xe^T @ A^T in bf16, f32 PSUM accum,
  - output partials stored as bf16.

ALL input DMAs are issued from the sync engine in one strict-priority
FIFO (inp tiles and early A^T chunks first): a single queue at full HBM
rate with the order we want beats parallel queues that round-robin at
packet granularity, and the sync engine runs no compute, so no
head-of-line blocking. Small transfers are paired into 0.5 MiB DMAs to
amortize the ~0.7us per-DMA issue cost.

The big matmul needs 16 PSUM-bank accumulators but TRN2 has 8 banks,
so banks are time-shared in three stages with partial parking:
  stage 1: (b,h)-chunks 0-1 accumulate e-chunks 0-3 as they arrive,
           then the 8 partials park to SBUF (f32), freeing the banks;
  stage 2: (b,h)-chunks 2-3 run their full 8-chunk chains (e-chunks
           0-3 replay dense from SBUF while 4-7 stream in);
  stage 3: (b,h)-chunks 0-1 accumulate e-chunks 4-7; the parked
           partial is added back during the PSUM->SBUF evacuation
           (DVE tensor_tensor, the only engine that can).
This leaves only 32 (not 64) matmuls strictly after the last chunk.

Late mask-multiplies (e-chunks 4-7) are split in halves so the first
half of the last chunk reaches the PE ~1.2us earlier.
"""

from contextlib import ExitStack

import ml_dtypes
import numpy as np

import concourse.bass as bass
import concourse.mybir as mybir
import concourse.tile as tile
from concourse import bacc
from concourse.bass_utils import run_bass_kernel_spmd

B, N, E, DIM = 16, 2048, 8192, 64
DH = DIM // 2              # 32
NCORES = 8
EC = E // NCORES           # 1024 edges per core
KC = EC // 128             # 8 e-chunks of 128
BH = B * DH                # 512 (flattened (b, h) output dim)
NB = N // 512              # 4 column blocks of the big matmul
NJ = B // 2                # 8 input tiles, two batch rows packed per tile
KSPLIT = KC // 2           # stage-1/stage-3 split of the contraction

F32 = mybir.dt.float32
BF16 = mybir.dt.bfloat16
U8 = mybir.dt.uint8
BF16NP = ml_dtypes.bfloat16

_PROGRAMS: dict = {}


def _build_program(with_bxes: bool, with_b: bool):
    nc = bacc.Bacc(
        "TRN2", target_bir_lowering=False, debug=False, enable_asserts=False
    )

    # inp pairs: [pair, 128, 2, EC] so each 0.5 MiB DMA is contiguous
    inp_t = nc.dram_tensor(
        "inp_t", [NJ // 2, 128, 2, EC], BF16, kind="ExternalInput"
    ).ap()
    wq = nc.dram_tensor("wq", [KC, 128, N], BF16, kind="ExternalInput").ap()
    iq = nc.dram_tensor(
        "iq", [KC // 2, 128, 2, N], U8, kind="ExternalInput"
    ).ap()
    wx = nc.dram_tensor("wx", [128, 2 * DH], BF16, kind="ExternalInput").ap()
    bxr = (
        nc.dram_tensor("bxr", [128, BH], F32, kind="ExternalInput").ap()
        if with_bxes
        else None
    )
    bq = (
        nc.dram_tensor("bq", [KC, 128, N], BF16, kind="ExternalInput").ap()
        if with_b
        else None
    )
    outp = nc.dram_tensor("outp", [BH, N], BF16, kind="ExternalOutput").ap()

    with tile.TileContext(nc) as tc, ExitStack() as ctx:
        inp_pool = ctx.enter_context(tc.tile_pool(name="inp", bufs=NJ // 2))
        wx_pool = ctx.enter_context(tc.tile_pool(name="wx", bufs=1))
        xe_pool = ctx.enter_context(tc.tile_pool(name="xe", bufs=KC))
        a_pool = ctx.enter_context(tc.tile_pool(name="a", bufs=KC))
        i_pool = ctx.enter_context(tc.tile_pool(name="i", bufs=KC // 2))
        park_pool = ctx.enter_context(tc.tile_pool(name="pk", bufs=2 * NB))
        out_pool = ctx.enter_context(tc.tile_pool(name="o", bufs=4))
        ps_pool = ctx.enter_context(tc.tile_pool(name="ps", bufs=8, space="PSUM"))

        # Block-diagonal xes weight: rows 0-63 map the even batch row to
        # output cols 0-31, rows 64-127 map the odd batch row to cols
        # 32-63, so one K=128 matmul computes xe for both packed batch
        # rows of an input tile at once.
        wx_tile = wx_pool.tile([128, 2 * DH], BF16)
        nc.sync.dma_start(wx_tile[:], wx[:])

        bx_tile = None
        if with_bxes:
            bx_tile = wx_pool.tile([128, BH], F32, tag="bx")
            nc.sync.dma_start(bx_tile[:], bxr[:])

        # ---- HAM warmup: matmul burst during the initial DMA dead time
        # (only wx has landed) so the PE clock-gate is at 8/8 when the
        # real stream starts. Scratch region of the bank xe chunk 7
        # reuses much later.
        ps_warm = ps_pool.tile([128, BH], F32, tag="ps", name="ps_warm")
        for i in range(16):
            nc.tensor.matmul(
                ps_warm[0:64, 0:64],
                wx_tile[:, 0:64],
                wx_tile[:, 0:64],
                start=True,
                stop=True,
            )

        # ---- tiles
        inp_tiles = [
            inp_pool.tile([128, 2, EC], BF16, tag="inp", name=f"inp_{p}",
                          bufs=NJ // 2)
            for p in range(NJ // 2)
        ]
        a_tiles = [
            a_pool.tile([128, N], BF16, tag="a", name=f"a_{k}", bufs=KC)
            for k in range(KC)
        ]
        it_tiles = [
            i_pool.tile([128, 2, N], U8, tag="it", name=f"it_{p}", bufs=KC // 2)
            for p in range(KC // 2)
        ]

        # ---- loads: ONE priority-ordered FIFO on the sync queue.
        nc.sync.dma_start(inp_tiles[0][:], inp_t[0])
        nc.sync.dma_start(inp_tiles[1][:], inp_t[1])
        nc.sync.dma_start(a_tiles[0][:], wq[0])
        nc.sync.dma_start(inp_tiles[2][:], inp_t[2])
        nc.sync.dma_start(it_tiles[0][:], iq[0])      # inci chunks 0-1
        nc.sync.dma_start(inp_tiles[3][:], inp_t[3])
        nc.sync.dma_start(a_tiles[1][:], wq[1])
        nc.sync.dma_start(it_tiles[1][:], iq[1])      # inci chunks 2-3
        nc.sync.dma_start(a_tiles[2][:], wq[2])
        nc.sync.dma_start(a_tiles[3][:], wq[3])
        nc.sync.dma_start(it_tiles[2][:], iq[2])      # inci chunks 4-5
        nc.sync.dma_start(a_tiles[4][:], wq[4])
        nc.sync.dma_start(a_tiles[5][:], wq[5])
        nc.sync.dma_start(it_tiles[3][:], iq[3])      # inci chunks 6-7
        nc.sync.dma_start(a_tiles[6][:], wq[6])
        nc.sync.dma_start(a_tiles[7][:], wq[7])

        # ---- xe = relu(inputs @ W_xes) in [e, (b,h)] layout.
        # One PSUM bank per e-chunk; walk j outermost so each input pair
        # is consumed as soon as its DMA lands.
        ps_xe = [
            ps_pool.tile([128, BH], F32, tag="ps", name=f"ps_xe_{k}")
            for k in range(KC)
        ]
        for j in range(NJ):
            lhs_src = inp_tiles[j // 2]
            for k in range(KC):
                nc.tensor.matmul(
                    ps_xe[k][:, j * 2 * DH : (j + 1) * 2 * DH],
                    lhs_src[:, j % 2, k * 128 : (k + 1) * 128],
                    wx_tile[:],
                    start=True,
                    stop=True,
                )
        xe_tiles = []
        for k in range(KC):
            xt = xe_pool.tile([128, BH], BF16)
            if with_bxes:
                nc.vector.tensor_tensor(
                    xt[:], ps_xe[k][:], bx_tile[:], op=mybir.AluOpType.add
                )
                nc.scalar.activation(
                    xt[:], xt[:], mybir.ActivationFunctionType.Relu
                )
            else:
                nc.scalar.activation(
                    xt[:], ps_xe[k][:], mybir.ActivationFunctionType.Relu
                )
            xe_tiles.append(xt)

        # ---- A^T chunks: a[k] *= inci[k] on the DVE (bf16 x u8 mixed
        # dtype). Late chunks split in halves to cut PE wait latency.
        def emit_mult(k):
            ipair = it_tiles[k // 2]
            if k < KSPLIT:
                nc.vector.tensor_tensor(
                    a_tiles[k][:], a_tiles[k][:], ipair[:, k % 2],
                    op=mybir.AluOpType.mult,
                )
            else:
                for hlf in range(2):
                    sl = slice(hlf * (N // 2), (hlf + 1) * (N // 2))
                    nc.vector.tensor_tensor(
                        a_tiles[k][:, sl], a_tiles[k][:, sl],
                        ipair[:, k % 2, sl],
                        op=mybir.AluOpType.mult,
                    )
            if with_b:
                bt = i_pool.tile([128, N], BF16, tag="bt", bufs=2)
                nc.sync.dma_start(bt[:], bq[k])
                nc.vector.tensor_tensor(
                    a_tiles[k][:], a_tiles[k][:], bt[:],
                    op=mybir.AluOpType.add,
                )

        for k in range(KC):
            emit_mult(k)

        # ---- big matmul: out[(b,h), n] += xe^T @ A^T, bf16, f32 accum.

        # stage 1: (b,h)-chunks 0-1, e-chunks 0..KSPLIT-1 as they arrive.
        ps1 = [
            [
                ps_pool.tile([128, 512], F32, tag="ps", name=f"ps1_{h}_{nb}")
                for nb in range(NB)
            ]
            for h in range(2)
        ]
        for k in range(KSPLIT):
            for h in range(2):
                lhsT = xe_tiles[k][:, h * 128 : (h + 1) * 128]
                for nb in range(NB):
                    nc.tensor.matmul(
                        ps1[h][nb][:],
                        lhsT,
                        a_tiles[k][:, nb * 512 : (nb + 1) * 512],
                        start=(k == 0),
                        stop=(k == KSPLIT - 1),
                    )
        # park the 8 stage-1 partials to SBUF (f32) on ScalarE, freeing
        # the banks (stage-3 adds them back on the DVE at evacuation).
        park = [[None] * NB for _ in range(2)]
        for h in range(2):
            for nb in range(NB):
                pk = park_pool.tile([128, 512], F32, tag="pk", name=f"pk_{h}_{nb}")
                nc.scalar.activation(
                    pk[:], ps1[h][nb][:],
                    mybir.ActivationFunctionType.Identity,
                )
                park[h][nb] = pk

        # stage 2: (b,h)-chunks 2-3 full chains; e-chunks 0..3 replay
        # dense from SBUF, 4..7 consumed as they arrive. nb-major order
        # inside each k so the half-split multiplies unlock MMs sooner.
        ps2 = [
            [
                ps_pool.tile([128, 512], F32, tag="ps", name=f"ps2_{h}_{nb}")
                for nb in range(NB)
            ]
            for h in range(2)
        ]
        for k in range(KC):
            for nb in range(NB):
                for h in range(2):
                    nc.tensor.matmul(
                        ps2[h][nb][:],
                        xe_tiles[k][:, (h + 2) * 128 : (h + 3) * 128],
                        a_tiles[k][:, nb * 512 : (nb + 1) * 512],
                        start=(k == 0),
                        stop=(k == KC - 1),
                    )
        for h in range(2):
            ot = out_pool.tile([128, N], BF16, tag="o", name=f"ot2_{h}")
            for nb in range(NB):
                nc.scalar.activation(
                    ot[:, nb * 512 : (nb + 1) * 512],
                    ps2[h][nb][:],
                    mybir.ActivationFunctionType.Identity,
                )
            nc.scalar.dma_start(outp[(h + 2) * 128 : (h + 3) * 128, :], ot[:])

        # stage 3: (b,h)-chunks 0-1, e-chunks KSPLIT..KC-1; the parked
        # stage-1 partial is added back during evacuation (DVE).
        ps3 = [
            [
                ps_pool.tile([128, 512], F32, tag="ps", name=f"ps3_{h}_{nb}")
                for nb in range(NB)
            ]
            for h in range(2)
        ]
        for k in range(KSPLIT, KC):
            for h in range(2):
                lhsT = xe_tiles[k][:, h * 128 : (h + 1) * 128]
                for nb in range(NB):
                    nc.tensor.matmul(
                        ps3[h][nb][:],
                        lhsT,
                        a_tiles[k][:, nb * 512 : (nb + 1) * 512],
                        start=(k == KSPLIT),
                        stop=(k == KC - 1),
                    )
        for h in range(2):
            ot = out_pool.tile([128, N], BF16, tag="o", name=f"ot3_{h}")
            for nb in range(NB):
                nc.vector.tensor_tensor(
                    ot[:, nb * 512 : (nb + 1) * 512],
                    ps3[h][nb][:],
                    park[h][nb][:],
                    op=mybir.AluOpType.add,
                )
            nc.sync.dma_start(outp[h * 128 : (h + 1) * 128, :], ot[:])

    nc.compile()
    return nc


def _get_program(with_bxes: bool, with_b: bool):
    key = (with_bxes, with_b)
    if key not in _PROGRAMS:
        _PROGRAMS[key] = _build_program(with_bxes, with_b)
    return _PROGRAMS[key]


def _prepare_in_maps(inputs, W_xes, b_xes, inci, w, b, with_bxes, with_b):
    inputs = np.asarray(inputs, dtype=np.float32)
    W_xes = np.asarray(W_xes, dtype=np.float32)
    b_xes = np.asarray(b_xes, dtype=np.float32)
    w = np.asarray(w, dtype=np.float32)
    b = np.asarray(b, dtype=np.float32)
    inci_u8 = np.asarray(inci).astype(np.uint8)

    wx_dup = np.zeros((128, 2 * DH), dtype=np.float32)
    wx_dup[0:DIM, 0:DH] = W_xes
    wx_dup[DIM : 2 * DIM, DH : 2 * DH] = W_xes
    wx_dup = wx_dup.astype(BF16NP)
    bxr = np.ascontiguousarray(
        np.broadcast_to(np.tile(b_xes, B)[None, :], (128, BH))
    ) if with_bxes else None

    in_maps = []
    for c in range(NCORES):
        sl = slice(c * EC, (c + 1) * EC)
        # [NJ, 128, EC] then paired -> [NJ//2, 128, 2, EC]
        t = np.ascontiguousarray(
            inputs[:, sl, :].transpose(0, 2, 1)
        ).reshape(NJ, 128, EC).astype(BF16NP)
        t = np.ascontiguousarray(t.reshape(NJ // 2, 2, 128, EC).transpose(0, 2, 1, 3))
        wq_ = np.ascontiguousarray(w[:, sl].T).reshape(KC, 128, N).astype(BF16NP)
        iq_ = np.ascontiguousarray(inci_u8[:, sl].T).reshape(KC, 128, N)
        iq_ = np.ascontiguousarray(
            iq_.reshape(KC // 2, 2, 128, N).transpose(0, 2, 1, 3)
        )
        m = {"inp_t": t, "wq": wq_, "iq": iq_, "wx": wx_dup}
        if with_bxes:
            m["bxr"] = bxr
        if with_b:
            m["bq"] = np.ascontiguousarray(b[:, sl].T).reshape(
                KC, 128, N
            ).astype(BF16NP)
        in_maps.append(m)
    return in_maps


def _run(inputs, W_xes, b_xes, inci, w, b, **run_kwargs):
    with_bxes = bool(np.any(np.asarray(b_xes)))
    with_b = bool(np.any(np.asarray(b)))
    nc = _get_program(with_bxes, with_b)
    in_maps = _prepare_in_maps(inputs, W_xes, b_xes, inci, w, b, with_bxes, with_b)
    res = run_bass_kernel_spmd(
        nc, in_maps, core_ids=list(range(NCORES)), **run_kwargs
    )
    parts = np.stack(
        [r["outp"].astype(np.float32) for r in res.results]
    )  # [8, BH, N]
    out = parts.sum(axis=0)  # [BH, N]
    out = out.reshape(B, DH, N).transpose(0, 2, 1)  # [B, N, DH]
    return np.ascontiguousarray(out.astype(np.float32)), res


def kernel(inputs, W_xes, b_xes, inci, w, b):
    out, _ = _run(inputs, W_xes, b_xes, inci, w, b)
    return out
